# revision 1
# baseline (speedup 1.0000x reference)
"""Trainium2 Bass kernel for causal self-attention (dense transformer block attn).

Reference computation (per batch b):
    qkv = x @ W_attn + b_attn ; split into per-head Q, K, V (16 heads, hs=64)
    att = softmax(mask(Q K^T / sqrt(hs))) ; y = att @ V ; out = y @ W_proj + b_proj

Sharding (8 cores): data parallel on B (2) x tensor parallel on head groups
(4 groups of 4 heads, Megatron-style column/row split of W_attn / W_proj).
Each core computes a partial out^T [1024, 2048] (bf16); host sums the 4
partials per batch, adds b_proj and transposes.

Core kernel layout notes:
  - Everything on-chip is transposed: x^T, qkv^T ([feature, T]), scores are
    computed as S^T = K Q^T with k-positions on partitions so that the PV
    matmul needs no transposes (P^T is the moving operand, V natural the
    stationary).
  - Emission is flash-style (outer loop over 512-wide q blocks, inner over
    128-wide k chunks) so score/exp/PV/proj work for early q blocks
    completes while later scores are still streaming; PV / normalize / proj
    lag their producers by roughly one q block so the in-order engine
    queues never wait on fresh cross-engine results.
  - Softmax denominator: the PV stationary is [V | ones] (or [ones | V]) so
    the complementary 64 psum partitions accumulate copies of
    sum_k P[q,k]; a single-row reciprocal_approx_fast + a DRAM partition
    broadcast bounce (on the otherwise idle gpsimd SWDGE queue) yields the
    per-q scale; one DVE multiply per head normalizes during evacuation.
  - exp() runs on ScalarE straight out of PSUM in wide [128, 2, <=512]
    instructions (two heads at once) to amortize the ~352-cycle ACT
    overhead.
"""

import numpy as np
import ml_dtypes

import concourse.bass as bass
import concourse.tile as tile
import concourse.mybir as mybir
from concourse import bacc
from concourse.bass_utils import run_bass_kernel_spmd

BF16 = mybir.dt.bfloat16
F32 = mybir.dt.float32
AF = mybir.ActivationFunctionType

T = 2048          # sequence length
C = 1024          # model dim
HPC = 4           # heads per core
HS = 64           # head size
NF = 3 * HPC * HS  # per-core qkv features (768)
N_CORES = 8
QB = 512          # q block (psum bank of f32)

bf16 = ml_dtypes.bfloat16


def build_kernel():
    nc = bacc.Bacc("TRN2", target_bir_lowering=False, debug=False)

    xT = nc.dram_tensor("xT", [C, T], BF16, kind="ExternalInput").ap()
    W = nc.dram_tensor("W", [C, NF], BF16, kind="ExternalInput").ap()
    bcols = nc.dram_tensor("bcols", [128, 6], F32, kind="ExternalInput").ap()
    Wp = nc.dram_tensor("Wp", [HPC * HS, C], BF16, kind="ExternalInput").ap()
    mask = nc.dram_tensor("mask", [128, 128], BF16, kind="ExternalInput").ap()
    ident = nc.dram_tensor("ident", [128, 128], BF16, kind="ExternalInput").ap()
    outT = nc.dram_tensor("outT", [C, T], BF16, kind="ExternalOutput").ap()

    with tile.TileContext(nc) as tc:
        _emit(nc, tc, xT, W, bcols, Wp, mask, ident, outT)
    nc.compile()
    return nc


def _emit(nc, tc, xT, W, bcols, Wp, mask, ident, outT):
    from contextlib import ExitStack

    ctx = ExitStack()
    consts = ctx.enter_context(tc.tile_pool(name="consts", bufs=1))
    pt_pool = ctx.enter_context(tc.tile_pool(name="pt", bufs=1))
    rt_pool = ctx.enter_context(tc.tile_pool(name="rt", bufs=2))
    osb_pool = ctx.enter_context(tc.tile_pool(name="osb", bufs=2))
    ps_s = ctx.enter_context(tc.tile_pool(name="ps_s", bufs=2, space="PSUM"))
    ps_sm = ctx.enter_context(tc.tile_pool(name="ps_sm", bufs=2, space="PSUM"))
    ps_junk = ctx.enter_context(tc.tile_pool(name="ps_junk", bufs=1, space="PSUM"))

    # ---------------- constant / input loads ----------------
    # x and W interleaved per c-chunk with x split in T halves so the first
    # q blocks are available early; big/first loads on the Sync HWDGE queue,
    # second x half on the Scalar HWDGE queue, small consts on gpsimd SWDGE.
    xT_v = xT.rearrange("(c p) t -> p c t", p=128)
    xT_t = consts.tile([128, 8, T], BF16, tag="xT", name="xT_t")
    W_v = W.rearrange("(c p) n -> p c n", p=128)
    W_t = consts.tile([128, 8, NF], BF16, tag="W", name="W_t")
    TH = T // 2
    for c in range(8):
        nc.sync.dma_start(out=W_t[:, c, :], in_=W_v[:, c, :])
        nc.sync.dma_start(out=xT_t[:, c, 0:TH], in_=xT_v[:, c, 0:TH])
        nc.scalar.dma_start(out=xT_t[:, c, TH:T], in_=xT_v[:, c, TH:T])
    b_t = consts.tile([128, 6], F32, tag="b", name="b_t")
    nc.gpsimd.dma_start(out=b_t, in_=bcols)
    Wp_t = consts.tile([128, 2, C], BF16, tag="Wp", name="Wp_t")
    nc.gpsimd.dma_start(out=Wp_t, in_=Wp.rearrange("(k p) n -> p k n", p=128))
    mask_t = consts.tile([128, 128], BF16, tag="mask", name="mask_t")
    nc.gpsimd.dma_start(out=mask_t, in_=mask)
    id_t = consts.tile([128, 128], BF16, tag="ident", name="id_t")
    nc.gpsimd.dma_start(out=id_t, in_=ident)

    # DRAM scratch for the reciprocal broadcast bounce: [pair*qb, hl, 512]
    scr = nc.dram_tensor("pv_scr", [8, 2, QB], F32).ap()

    qkvT = consts.tile([128, 6, T], BF16, tag="qkvT", name="qkvT")
    # vnat[p, pair, j, hl, col]: PV stationary tiles. hl=0: [V | ones],
    # hl=1: [ones | V] so that y lands on the partitions matching yT layout.
    vnat = consts.tile([128, 2, 16, 2, 128], BF16, tag="vnat", name="vnat")
    yT = consts.tile([128, 2, T], BF16, tag="yT", name="yT")

    # warm up the ACT exp table early so the ~2.7us load overlaps the lead-in
    warm = consts.tile([128, 8], F32, tag="warm", name="warm")
    nc.vector.memset(warm, 0.0)
    nc.scalar.activation(warm, warm, AF.Exp, scale=1.0)

    # input-DMA-independent junk matmuls: keep the PE array fed during the
    # initial input-streaming window so HAM un-throttles before real work
    jw = consts.tile([128, QB], BF16, tag="jw", name="jw")
    nc.vector.memset(jw, 0.0)
    junk = ps_junk.tile([128, QB], F32, tag="junk", name="junk")

    def keep_warm(n=2):
        for _ in range(n):
            nc.tensor.matmul(junk, lhsT=jw[:, 0:128], rhs=jw, start=True,
                             stop=True)

    nc.vector.memset(vnat[:, :, :, 0, 64:128], 1.0)
    nc.vector.memset(vnat[:, :, :, 1, 0:64], 1.0)

    # ---------------- phase helpers ----------------
    def qkv_part(nf, qb4):
        # one q block of qkv^T[nf*128:(nf+1)*128, :]  (+ bias on evac)
        ps = ps_sm.tile([128, QB], F32, tag="sm", name="ps_qkv")
        for c in range(8):
            nc.tensor.matmul(
                ps,
                lhsT=W_t[:, c, nf * 128:(nf + 1) * 128],
                rhs=xT_t[:, c, qb4 * QB:(qb4 + 1) * QB],
                start=(c == 0),
                stop=(c == 7),
            )
        nc.vector.tensor_scalar_add(
            qkvT[:, nf, qb4 * QB:(qb4 + 1) * QB], ps, b_t[:, nf:nf + 1]
        )

    def vtrans_chunk(p, jc):
        # V^T tile chunk jc (qkvT[:, 4+p, jc]) -> natural V in vnat[:, p, jc]
        pst = ps_sm.tile([128, 128], BF16, tag="vt", name="ps_vt", bufs=1)
        nc.tensor.transpose(pst, qkvT[:, 4 + p, jc * 128:(jc + 1) * 128], id_t)
        # single strided copy: psum cols [0:64|64:128] -> vnat
        # [jc, 0, 0:64] and [jc, 1, 64:128]
        v0 = vnat[:, p, jc, 0, 0:64]
        dst = bass.AP(tensor=v0.tensor, offset=v0.offset,
                      ap=[v0.ap[0], [192, 2], [1, 64]])
        s0 = pst[:, 0:64]
        src = bass.AP(tensor=s0.tensor, offset=s0.offset,
                      ap=[s0.ap[0], [64, 2], [1, 64]])
        nc.vector.tensor_copy(dst, src)

    pt_tiles = {}

    def s_part(p, j, qb4):
        # scores^T + exp for pair p, key chunk j, q block qb4 (both heads)
        wj = T - 128 * j
        if (p, j) not in pt_tiles:
            pt_tiles[(p, j)] = pt_pool.tile(
                [128, 2, wj], BF16, tag=f"pt{j}",
                name=f"pt_{p}_{j}", bufs=2 if j < 2 else 1)
        pt = pt_tiles[(p, j)]
        qlo = max(128 * j, QB * qb4)
        qhi = QB * (qb4 + 1)
        lo = qlo - QB * qb4
        ps = ps_s.tile([128, 2, QB], F32, tag="s", name="ps_s_t")
        for hl in range(2):
            nc.tensor.matmul(
                ps[:, hl, lo:QB],
                lhsT=qkvT[64 * hl:64 * hl + 64, 2 + p, j * 128:(j + 1) * 128],
                rhs=qkvT[64 * hl:64 * hl + 64, p, qlo:qhi],
                start=True,
                stop=True,
            )
        nc.scalar.activation(
            pt[:, :, (qlo - 128 * j):(qhi - 128 * j)],
            ps[:, :, lo:QB],
            AF.Exp,
            scale=0.125,
        )
        if j // 4 == qb4:
            # zero the q < k upper triangle of the diagonal chunk (both heads
            # in one mul via a broadcast AP over the head dim)
            mb = bass.AP(tensor=mask_t.tensor, offset=mask_t.offset,
                         ap=[mask_t.ap[0], [0, 2], [1, 128]])
            nc.vector.tensor_mul(pt[:, :, 0:128], pt[:, :, 0:128], mb)

    sb_tiles = {}
    rt2_tiles = {}

    def pv_unit(p, hl, qb4):
        # y^T (and denominator copies) for head (p, hl), q block qb4:
        # matmul chain + psum evac + single-row reciprocal + scratch dump.
        drow = 64 - 64 * hl  # one representative denominator-copy row
        ps = ps_sm.tile([128, QB], F32, tag="sm", name=f"ps_pv{p}{hl}")
        last = 4 * qb4 + 3
        for jp in range(0, last + 1):
            pt = pt_tiles[(p, jp)]
            qlo = max(qb4 * QB, 128 * jp)
            qhi = qb4 * QB + QB
            nc.tensor.matmul(
                ps[:, (qlo - qb4 * QB):(qhi - qb4 * QB)],
                lhsT=vnat[:, p, jp, hl, :],
                rhs=pt[:, hl, (qlo - 128 * jp):(qhi - 128 * jp)],
                start=(jp == 0),
                stop=(jp == last),
            )
        # One fast copy frees the psum bank; then dump one raw
        # denominator-copy row to DRAM scratch (idle gpsimd SWDGE queue).
        sb = rt_pool.tile([128, QB], F32, tag="sb", name="sb", bufs=6)
        nc.vector.tensor_copy(sb, ps)
        uid = p * 4 + qb4
        nc.gpsimd.dma_start(out=scr[uid, hl, :], in_=sb[drow:drow + 1, :])
        sb_tiles[(p, hl, qb4)] = sb

    def pv2(p, qb4):
        # both heads' PV chains plus one partition-broadcast read for the
        # pair: rt2 rows 0:64 get hl0's reciprocal row, 64:128 get hl1's.
        pv_unit(p, 0, qb4)
        pv_unit(p, 1, qb4)
        uid = p * 4 + qb4
        s1 = scr[uid]
        src = bass.AP(tensor=s1.tensor, offset=s1.offset,
                      ap=[[QB, 2], [0, 64], [1, QB]])
        rt2 = rt_pool.tile([128, QB], F32, tag="rt2", name="rt2", bufs=3)
        # the 256KB broadcast must go on a HWDGE queue: the single-ring
        # gpsimd SWDGE queue takes ~7us for this transfer vs ~1us fanned out
        nc.sync.dma_start(out=rt2, in_=src)
        rt2_tiles[(p, qb4)] = rt2

    def pv_norm(p, qb4):
        # normalize both heads' y into yT. The reciprocal happens here, one
        # emission block after pv2 issued the broadcast DMA, so the in-order
        # DVE queue never stalls waiting for it (full-tile approx covers
        # both heads at once; single-partition reciprocal_approx is broken).
        qsl = slice(qb4 * QB, (qb4 + 1) * QB)
        rt2 = rt2_tiles.pop((p, qb4))
        rc = rt_pool.tile([128, QB], F32, tag="rc", name="rc", bufs=2)
        nc.vector.reciprocal_approx_fast(out=rc, in_=rt2)
        for hl in range(2):
            ysl = slice(64 * hl, 64 * hl + 64)
            sb = sb_tiles.pop((p, hl, qb4))
            nc.vector.tensor_mul(yT[ysl, p, qsl], sb[ysl, :], rc[ysl, :])

    outT_v = outT.rearrange("(n p) t -> p n t", p=128)

    def proj_qb(qb4, evac_engine):
        # final projection for one q block (needs yT of both pairs for it)
        qsl = slice(qb4 * QB, (qb4 + 1) * QB)
        for nf2 in range(4):
            ob = osb_pool.tile([128, 2, QB], BF16, tag="osb", name="ob")
            for sub in range(2):
                nf = nf2 * 2 + sub
                ps = ps_sm.tile([128, QB], F32, tag="sm", name="ps_o")
                for kc in range(2):
                    nc.tensor.matmul(
                        ps,
                        lhsT=Wp_t[:, kc, nf * 128:(nf + 1) * 128],
                        rhs=yT[:, kc, qsl],
                        start=(kc == 0),
                        stop=(kc == 1),
                    )
                if evac_engine == "scalar":
                    nc.scalar.copy(ob[:, sub, :], ps)
                else:
                    nc.vector.tensor_copy(ob[:, sub, :], ps)
            nc.sync.dma_start(out=outT_v[:, nf2 * 2:nf2 * 2 + 2, qsl], in_=ob)

    # ---------------- emission schedule ----------------
    # flash-style: per 512-wide q block of pair 0 then pair 1: QKV feeders
    # first, then scores+exp for all k chunks <= the diagonal with the
    # carry-over work (V transposes, lagged PV/normalize/proj of earlier
    # blocks) interleaved between score steps as PE filler.
    def iteration(p, qb4, fillers, warm_every=0):
        qkv_part(p, qb4)       # Q_p for this block
        qkv_part(2 + p, qb4)   # K_p chunks 4qb..4qb+3
        fill = list(fillers)
        nf_s = 4 * qb4 + 4
        for j in range(nf_s):
            s_part(p, j, qb4)
            if warm_every:
                keep_warm(warm_every)
            # spread fillers across the score steps, back-loaded so lagged
            # PV/proj work lands late in the iteration
            while fill and len(fill) > (nf_s - 1 - j) * len(fillers) // nf_s:
                fill.pop(0)()
        for f in fill:
            f()

    def F(fn, *a):
        return lambda: fn(*a)

    def vts(p, j0):
        return [F(vtrans_chunk, p, j0 + i) for i in range(4)]

    # pv2(prev) goes FIRST in each iteration so its broadcast DMA is in
    # flight ~12us before pv_norm(prev) consumes it at the iteration's END
    # (after every DVE op the PE depends on). The V pipeline (qkv V part +
    # transposes) runs in its own block's iteration.
    with nc.named_scope("p0"):
        iteration(0, 0, [F(qkv_part, 4, 0)] + vts(0, 0), warm_every=2)
        iteration(0, 1, [F(pv2, 0, 0), F(qkv_part, 4, 1)] + vts(0, 4)
                  + [F(pv_norm, 0, 0)])
        iteration(0, 2, [F(pv2, 0, 1), F(qkv_part, 4, 2)] + vts(0, 8)
                  + [F(pv_norm, 0, 1)])
        iteration(0, 3, [F(pv2, 0, 2), F(qkv_part, 4, 3)] + vts(0, 12)
                  + [F(pv_norm, 0, 2)])
    with nc.named_scope("p1"):
        iteration(1, 0, [F(pv2, 0, 3), F(qkv_part, 5, 0)] + vts(1, 0)
                  + [F(pv_norm, 0, 3)])
        iteration(1, 1, [F(pv2, 1, 0), F(qkv_part, 5, 1)] + vts(1, 4)
                  + [F(pv_norm, 1, 0)])
        iteration(1, 2, [F(pv2, 1, 1), F(qkv_part, 5, 2)] + vts(1, 8)
                  + [F(pv_norm, 1, 1), F(proj_qb, 0, "vector")])
        iteration(1, 3, [F(pv2, 1, 2), F(qkv_part, 5, 3)] + vts(1, 12)
                  + [F(pv_norm, 1, 2), F(proj_qb, 1, "vector")])
    with nc.named_scope("tail"):
        pv2(1, 3)
        with nc.named_scope("prj2"):
            proj_qb(2, "scalar")
        pv_norm(1, 3)
        with nc.named_scope("prj3"):
            proj_qb(3, "scalar")
    ctx.close()


# ---------------------------------------------------------------------------
# host-side wrapper
# ---------------------------------------------------------------------------

_NC_CACHE = {}


def _get_nc():
    if "nc" not in _NC_CACHE:
        _NC_CACHE["nc"] = build_kernel()
    return _NC_CACHE["nc"]


def make_in_maps(x, W_attn, b_attn, W_proj, b_proj):
    B = x.shape[0]
    # multiplicative causal mask for the diagonal chunk, [k, q]: 1 where q >= k
    mask_np = np.triu(np.ones((128, 128), np.float32)).astype(bf16)
    ident_np = np.eye(128, dtype=np.float32).astype(bf16)
    in_maps = []
    for core in range(N_CORES):
        b = core // 4
        g = core % 4
        cols = np.r_[256 * g:256 * g + 256,
                     1024 + 256 * g:1024 + 256 * g + 256,
                     2048 + 256 * g:2048 + 256 * g + 256]
        in_maps.append({
            "xT": np.ascontiguousarray(x[b].T).astype(bf16),
            "W": np.ascontiguousarray(W_attn[:, cols]).astype(bf16),
            "bcols": np.ascontiguousarray(
                b_attn[cols].reshape(6, 128).T).astype(np.float32),
            "Wp": np.ascontiguousarray(
                W_proj[256 * g:256 * g + 256, :]).astype(bf16),
            "mask": mask_np,
            "ident": ident_np,
        })
    return in_maps


def kernel(x, W_attn, b_attn, W_proj, b_proj, _trace=False, _trace_kwargs=None):
    x = np.asarray(x, np.float32)
    W_attn = np.asarray(W_attn, np.float32)
    b_attn = np.asarray(b_attn, np.float32)
    W_proj = np.asarray(W_proj, np.float32)
    b_proj = np.asarray(b_proj, np.float32)

    nc = _get_nc()
    in_maps = make_in_maps(x, W_attn, b_attn, W_proj, b_proj)
    res = run_bass_kernel_spmd(
        nc, in_maps, core_ids=list(range(N_CORES)), trace=_trace,
        **(_trace_kwargs or {}),
    )
    B = x.shape[0]
    out = np.zeros((B, T, C), np.float32)
    for core in range(N_CORES):
        b = core // 4
        out[b] += res.results[core]["outT"].T.astype(np.float32)
    out += b_proj[None, None, :]
    if _trace:
        kernel._last_results = res
    return out


if __name__ == "__main__":
    # smoke test: build only
    nc = build_kernel()
    print("built ok")



# revision 2
# speedup vs baseline: 1.0458x; 1.0458x over previous
"""Trainium2 Bass kernel for causal self-attention (dense transformer block attn).

Reference computation (per batch b):
    qkv = x @ W_attn + b_attn ; split into per-head Q, K, V (16 heads, hs=64)
    att = softmax(mask(Q K^T / sqrt(hs))) ; y = att @ V ; out = y @ W_proj + b_proj

Sharding (8 cores): data parallel on B (2) x tensor parallel on head groups
(4 groups of 4 heads, Megatron-style column/row split of W_attn / W_proj).
Each core computes a partial out^T [1024, 2048] (bf16); host sums the 4
partials per batch, adds b_proj and transposes.

Core kernel layout notes:
  - Everything on-chip is transposed: x^T, q/k^T ([feature, T]), scores are
    computed as S^T = K Q^T with k-positions on partitions so that the PV
    matmul needs no transposes (P^T is the moving operand, V natural the
    stationary).
  - V is produced directly in natural [key, feature] layout by swapping the
    matmul roles (stationary = x^T k-chunk, moving = W_v columns); its bias
    is a rank-1 matmul (ones[1,128] x bv[1,128]) prepended to the chain.
    This removes all PE transposes and their DVE evacuation copies.
  - Emission is flash-style (outer loop over 512-wide q blocks, inner over
    128-wide k chunks). The ACT exp() stream is the pacing resource, so the
    schedule works to never stall it: Q/K projections for block qb+1 and
    V-natural rounds run as PE fillers inside block qb, and PSUM pools are
    split (scores / qkv+proj / pv) so no iteration-boundary matmul ever
    waits on the previous iteration's DVE backlog (which also kept HAM
    re-throttling the PE clock).
  - Softmax denominator: the PV stationary is [V | ones] (or [ones | V]) so
    the complementary 64 psum partitions accumulate copies of
    sum_k P[q,k]; a single-row reciprocal_approx_fast + a DRAM partition
    broadcast bounce (on the otherwise idle gpsimd SWDGE queue) yields the
    per-q scale; one DVE multiply per head normalizes during evacuation.
  - exp() runs on ScalarE straight out of PSUM in wide [128, 2, <=512]
    instructions (two heads at once) to amortize the ~352-cycle ACT
    overhead.
"""

import numpy as np
import ml_dtypes

import concourse.bass as bass
import concourse.tile as tile
import concourse.mybir as mybir
from concourse import bacc
from concourse.bass_utils import run_bass_kernel_spmd

BF16 = mybir.dt.bfloat16
F32 = mybir.dt.float32
AF = mybir.ActivationFunctionType

T = 2048          # sequence length
C = 1024          # model dim
HPC = 4           # heads per core
HS = 64           # head size
NF = 3 * HPC * HS  # per-core qkv features (768)
N_CORES = 8
QB = 512          # q block (psum bank of f32)

bf16 = ml_dtypes.bfloat16


def build_kernel():
    nc = bacc.Bacc("TRN2", target_bir_lowering=False, debug=False)

    xT = nc.dram_tensor("xT", [C, T], BF16, kind="ExternalInput").ap()
    W = nc.dram_tensor("W", [C, NF], BF16, kind="ExternalInput").ap()
    bcols = nc.dram_tensor("bcols", [128, 4], F32, kind="ExternalInput").ap()
    bv = nc.dram_tensor("bv", [1, 256], BF16, kind="ExternalInput").ap()
    Wp = nc.dram_tensor("Wp", [HPC * HS, C], BF16, kind="ExternalInput").ap()
    mask = nc.dram_tensor("mask", [128, 128], BF16, kind="ExternalInput").ap()
    outT = nc.dram_tensor("outT", [C, T], BF16, kind="ExternalOutput").ap()

    with tile.TileContext(nc) as tc:
        _emit(nc, tc, xT, W, bcols, bv, Wp, mask, outT)
    nc.compile()
    return nc


def _emit(nc, tc, xT, W, bcols, bv, Wp, mask, outT):
    from contextlib import ExitStack

    ctx = ExitStack()
    consts = ctx.enter_context(tc.tile_pool(name="consts", bufs=1))
    pt_pool = ctx.enter_context(tc.tile_pool(name="pt", bufs=1))
    rt_pool = ctx.enter_context(tc.tile_pool(name="rt", bufs=2))
    osb_pool = ctx.enter_context(tc.tile_pool(name="osb", bufs=2))
    ps_s = ctx.enter_context(tc.tile_pool(name="ps_s", bufs=2, space="PSUM"))
    ps_sm = ctx.enter_context(tc.tile_pool(name="ps_sm", bufs=2, space="PSUM"))
    ps_pv = ctx.enter_context(tc.tile_pool(name="ps_pv", bufs=2, space="PSUM"))

    # ---------------- constant / input loads ----------------
    # x and W interleaved per c-chunk with x split in T halves so the first
    # q blocks are available early; big/first loads on the Sync HWDGE queue,
    # second x half on the Scalar HWDGE queue, small consts on gpsimd SWDGE.
    xT_v = xT.rearrange("(c p) t -> p c t", p=128)
    xT_t = consts.tile([128, 8, T], BF16, tag="xT", name="xT_t")
    W_v = W.rearrange("(c p) n -> p c n", p=128)
    W_t = consts.tile([128, 8, NF], BF16, tag="W", name="W_t")
    TH = T // 2
    for c in range(8):
        nc.sync.dma_start(out=W_t[:, c, :], in_=W_v[:, c, :])
        nc.sync.dma_start(out=xT_t[:, c, 0:TH], in_=xT_v[:, c, 0:TH])
        nc.scalar.dma_start(out=xT_t[:, c, TH:T], in_=xT_v[:, c, TH:T])
    b_t = consts.tile([128, 4], F32, tag="b", name="b_t")
    nc.gpsimd.dma_start(out=b_t, in_=bcols)
    bv_t = consts.tile([1, 256], BF16, tag="bv", name="bv_t")
    nc.gpsimd.dma_start(out=bv_t, in_=bv)
    Wp_t = consts.tile([128, 2, C], BF16, tag="Wp", name="Wp_t")
    nc.gpsimd.dma_start(out=Wp_t, in_=Wp.rearrange("(k p) n -> p k n", p=128))
    mask_t = consts.tile([128, 128], BF16, tag="mask", name="mask_t")
    nc.gpsimd.dma_start(out=mask_t, in_=mask)

    # DRAM scratch for the reciprocal broadcast bounce: [pair*qb, hl, 512]
    scr = nc.dram_tensor("pv_scr", [8, 2, QB], F32).ap()

    qkvT = consts.tile([128, 4, T], BF16, tag="qkvT", name="qkvT")
    # vnat[p, pair, jc, hl, col]: PV stationary tiles. hl=0: [V | ones],
    # hl=1: [ones | V] so that y lands on the partitions matching yT layout.
    vnat = consts.tile([128, 2, 16, 2, 128], BF16, tag="vnat", name="vnat")
    yT = consts.tile([128, 2, T], BF16, tag="yT", name="yT")
    ones1 = consts.tile([1, 128], BF16, tag="ones1", name="ones1")
    nc.vector.memset(ones1, 1.0)

    # warm up the ACT exp table early so the ~2.7us load overlaps the lead-in
    warm = consts.tile([128, 8], F32, tag="warm", name="warm")
    nc.vector.memset(warm, 0.0)
    nc.scalar.activation(warm, warm, AF.Exp, scale=1.0)

    # input-DMA-independent junk matmuls: keep the PE array fed during the
    # initial input-streaming window so HAM un-throttles before real work.
    # The junk psum shares the "pv" slots: all junk writes are emitted in
    # the first iteration, before any pv tile cycles onto its slot.
    jw = consts.tile([128, QB], BF16, tag="jw", name="jw")
    nc.vector.memset(jw, 0.0)
    junk = ps_pv.tile([128, QB], F32, tag="pv", name="junk")

    def keep_warm(n=2):
        for _ in range(n):
            nc.tensor.matmul(junk, lhsT=jw[:, 0:128], rhs=jw, start=True,
                             stop=True)

    nc.vector.memset(vnat[:, :, :, 0, 64:128], 1.0)
    nc.vector.memset(vnat[:, :, :, 1, 0:64], 1.0)

    # ---------------- phase helpers ----------------
    def qkv_part(nf, qb4):
        # one q block of q/k^T[nf*128:(nf+1)*128, :]  (+ bias on evac)
        ps = ps_sm.tile([128, QB], F32, tag="sm", name="ps_qkv")
        for c in range(8):
            nc.tensor.matmul(
                ps,
                lhsT=W_t[:, c, nf * 128:(nf + 1) * 128],
                rhs=xT_t[:, c, qb4 * QB:(qb4 + 1) * QB],
                start=(c == 0),
                stop=(c == 7),
            )
        nc.vector.tensor_scalar_add(
            qkvT[:, nf, qb4 * QB:(qb4 + 1) * QB], ps, b_t[:, nf:nf + 1]
        )

    def vnat_round(p, kc):
        # V natural for pair p, key chunk kc: [key, (hl0 hs | hl1 hs)].
        # rank-1 bias matmul + 8 c-chunk matmuls, then one strided copy
        # psum -> vnat[:, p, kc, hl, 64*hl : 64*hl+64].
        ps = ps_sm.tile([128, 128], F32, tag="sm", name="ps_vn")
        nc.tensor.matmul(ps, lhsT=ones1, rhs=bv_t[0:1, 128 * p:128 * p + 128],
                         start=True, stop=False)
        for c in range(8):
            nc.tensor.matmul(
                ps,
                lhsT=xT_t[:, c, kc * 128:(kc + 1) * 128],
                rhs=W_t[:, c, 512 + 128 * p:512 + 128 * p + 128],
                start=False,
                stop=(c == 7),
            )
        v0 = vnat[:, p, kc, 0, 0:64]
        dst = bass.AP(tensor=v0.tensor, offset=v0.offset,
                      ap=[v0.ap[0], [192, 2], [1, 64]])
        s0 = ps[:, 0:64]
        src = bass.AP(tensor=s0.tensor, offset=s0.offset,
                      ap=[s0.ap[0], [64, 2], [1, 64]])
        nc.vector.tensor_copy(dst, src)

    pt_tiles = {}

    def s_part(p, j, qb4):
        # scores^T + exp for pair p, key chunk j, q block qb4 (both heads)
        wj = T - 128 * j
        if (p, j) not in pt_tiles:
            pt_tiles[(p, j)] = pt_pool.tile(
                [128, 2, wj], BF16, tag=f"pt{j}",
                name=f"pt_{p}_{j}", bufs=2 if j < 2 else 1)
        pt = pt_tiles[(p, j)]
        qlo = max(128 * j, QB * qb4)
        qhi = QB * (qb4 + 1)
        lo = qlo - QB * qb4
        ps = ps_s.tile([128, 2, QB], F32, tag="s", name="ps_s_t")
        for hl in range(2):
            nc.tensor.matmul(
                ps[:, hl, lo:QB],
                lhsT=qkvT[64 * hl:64 * hl + 64, 2 + p, j * 128:(j + 1) * 128],
                rhs=qkvT[64 * hl:64 * hl + 64, p, qlo:qhi],
                start=True,
                stop=True,
            )
        nc.scalar.activation(
            pt[:, :, (qlo - 128 * j):(qhi - 128 * j)],
            ps[:, :, lo:QB],
            AF.Exp,
            scale=0.125,
        )
        if j // 4 == qb4:
            # zero the q < k upper triangle of the diagonal chunk (both heads
            # in one mul via a broadcast AP over the head dim)
            mb = bass.AP(tensor=mask_t.tensor, offset=mask_t.offset,
                         ap=[mask_t.ap[0], [0, 2], [1, 128]])
            nc.vector.tensor_mul(pt[:, :, 0:128], pt[:, :, 0:128], mb)

    sb_tiles = {}
    rt2_tiles = {}

    def pv_unit(p, hl, qb4):
        # y^T (and denominator copies) for head (p, hl), q block qb4:
        # matmul chain + psum evac + single-row scratch dump.
        drow = 64 - 64 * hl  # one representative denominator-copy row
        ps = ps_pv.tile([128, QB], F32, tag="pv", name=f"ps_pv{p}{hl}")
        last = 4 * qb4 + 3
        for jp in range(0, last + 1):
            pt = pt_tiles[(p, jp)]
            qlo = max(qb4 * QB, 128 * jp)
            qhi = qb4 * QB + QB
            nc.tensor.matmul(
                ps[:, (qlo - qb4 * QB):(qhi - qb4 * QB)],
                lhsT=vnat[:, p, jp, hl, :],
                rhs=pt[:, hl, (qlo - 128 * jp):(qhi - 128 * jp)],
                start=(jp == 0),
                stop=(jp == last),
            )
        # One fast copy frees the psum bank; then dump one raw
        # denominator-copy row to DRAM scratch (idle gpsimd SWDGE queue).
        sb = rt_pool.tile([128, QB], F32, tag="sb", name="sb", bufs=6)
        nc.vector.tensor_copy(sb, ps)
        uid = p * 4 + qb4
        nc.gpsimd.dma_start(out=scr[uid, hl, :], in_=sb[drow:drow + 1, :])
        sb_tiles[(p, hl, qb4)] = sb

    def pv2(p, qb4):
        # both heads' PV chains plus one partition-broadcast read for the
        # pair: rt2 rows 0:64 get hl0's reciprocal row, 64:128 get hl1's.
        pv_unit(p, 0, qb4)
        pv_unit(p, 1, qb4)
        uid = p * 4 + qb4
        s1 = scr[uid]
        src = bass.AP(tensor=s1.tensor, offset=s1.offset,
                      ap=[[QB, 2], [0, 64], [1, QB]])
        rt2 = rt_pool.tile([128, QB], F32, tag="rt2", name="rt2", bufs=3)
        # the 256KB broadcast must go on a HWDGE queue: the single-ring
        # gpsimd SWDGE queue takes ~7us for this transfer vs ~1us fanned out
        nc.sync.dma_start(out=rt2, in_=src)
        rt2_tiles[(p, qb4)] = rt2

    def pv_norm(p, qb4):
        # normalize both heads' y into yT. The reciprocal happens here, one
        # emission block after pv2 issued the broadcast DMA, so the in-order
        # DVE queue never stalls waiting for it (full-tile approx covers
        # both heads at once; single-partition reciprocal_approx is broken).
        qsl = slice(qb4 * QB, (qb4 + 1) * QB)
        rt2 = rt2_tiles.pop((p, qb4))
        rc = rt_pool.tile([128, QB], F32, tag="rc", name="rc", bufs=2)
        nc.vector.reciprocal_approx_fast(out=rc, in_=rt2)
        for hl in range(2):
            ysl = slice(64 * hl, 64 * hl + 64)
            sb = sb_tiles.pop((p, hl, qb4))
            nc.vector.tensor_mul(yT[ysl, p, qsl], sb[ysl, :], rc[ysl, :])

    outT_v = outT.rearrange("(n p) t -> p n t", p=128)

    def proj_qb(qb4, evac_engine):
        # final projection for one q block (needs yT of both pairs for it)
        qsl = slice(qb4 * QB, (qb4 + 1) * QB)
        for nf2 in range(4):
            ob = osb_pool.tile([128, 2, QB], BF16, tag="osb", name="ob")
            for sub in range(2):
                nf = nf2 * 2 + sub
                ps = ps_sm.tile([128, QB], F32, tag="sm", name="ps_o")
                for kc in range(2):
                    nc.tensor.matmul(
                        ps,
                        lhsT=Wp_t[:, kc, nf * 128:(nf + 1) * 128],
                        rhs=yT[:, kc, qsl],
                        start=(kc == 0),
                        stop=(kc == 1),
                    )
                if evac_engine == "scalar":
                    nc.scalar.copy(ob[:, sub, :], ps)
                else:
                    nc.vector.tensor_copy(ob[:, sub, :], ps)
            nc.sync.dma_start(out=outT_v[:, nf2 * 2:nf2 * 2 + 2, qsl], in_=ob)

    # ---------------- emission schedule ----------------
    # flash-style: per 512-wide q block of pair 0 then pair 1: scores+exp
    # for all k chunks <= the diagonal with the carry-over work (lagged
    # PV/normalize/proj of the previous block) and the LOOK-AHEAD work
    # (next block's Q/K projections, V-natural rounds) interleaved between
    # score steps as PE filler, so the exp stream never waits on a fresh
    # projection at an iteration boundary.
    def iteration(p, qb4, fillers, warm_every=0):
        fill = list(fillers)
        nf_s = 4 * qb4 + 4
        for j in range(nf_s):
            s_part(p, j, qb4)
            if warm_every:
                keep_warm(warm_every)
            # spread fillers across the score steps, back-loaded so lagged
            # PV/proj work lands late in the iteration
            while fill and len(fill) > (nf_s - 1 - j) * len(fillers) // nf_s:
                fill.pop(0)()
        for f in fill:
            f()

    def F(fn, *a):
        return lambda: fn(*a)

    def vns(p, k0):
        return [F(vnat_round, p, k0 + i) for i in range(4)]

    # pv2(prev) goes FIRST in each iteration so its broadcast DMA is in
    # flight ~12us before pv_norm(prev) consumes it at the iteration's END
    # (after every DVE op the PE depends on).
    with nc.named_scope("p0"):
        qkv_part(0, 0)   # Q_0 block 0
        qkv_part(2, 0)   # K_0 chunks 0..3
        iteration(0, 0, vns(0, 0) + [F(qkv_part, 0, 1), F(qkv_part, 2, 1)],
                  warm_every=2)
        iteration(0, 1, [F(pv2, 0, 0)] + vns(0, 4)
                  + [F(qkv_part, 0, 2), F(qkv_part, 2, 2), F(pv_norm, 0, 0)])
        iteration(0, 2, [F(pv2, 0, 1)] + vns(0, 8)
                  + [F(qkv_part, 0, 3), F(qkv_part, 2, 3), F(pv_norm, 0, 1)])
        iteration(0, 3, [F(pv2, 0, 2)] + vns(0, 12)
                  + [F(qkv_part, 1, 0), F(qkv_part, 3, 0), F(pv_norm, 0, 2)])
    with nc.named_scope("p1"):
        iteration(1, 0, [F(pv2, 0, 3)] + vns(1, 0)
                  + [F(qkv_part, 1, 1), F(qkv_part, 3, 1), F(pv_norm, 0, 3)])
        iteration(1, 1, [F(pv2, 1, 0)] + vns(1, 4)
                  + [F(qkv_part, 1, 2), F(qkv_part, 3, 2), F(pv_norm, 1, 0)])
        iteration(1, 2, [F(pv2, 1, 1)] + vns(1, 8)
                  + [F(qkv_part, 1, 3), F(qkv_part, 3, 3), F(pv_norm, 1, 1),
                     F(proj_qb, 0, "vector")])
        iteration(1, 3, [F(pv2, 1, 2)] + vns(1, 12)
                  + [F(pv_norm, 1, 2), F(proj_qb, 1, "vector")])
    with nc.named_scope("tail"):
        pv2(1, 3)
        with nc.named_scope("prj2"):
            proj_qb(2, "scalar")
        pv_norm(1, 3)
        with nc.named_scope("prj3"):
            proj_qb(3, "scalar")
    ctx.close()


# ---------------------------------------------------------------------------
# host-side wrapper
# ---------------------------------------------------------------------------

_NC_CACHE = {}


def _get_nc():
    if "nc" not in _NC_CACHE:
        _NC_CACHE["nc"] = build_kernel()
    return _NC_CACHE["nc"]


def make_in_maps(x, W_attn, b_attn, W_proj, b_proj):
    # multiplicative causal mask for the diagonal chunk, [k, q]: 1 where q >= k
    mask_np = np.triu(np.ones((128, 128), np.float32)).astype(bf16)
    in_maps = []
    for core in range(N_CORES):
        b = core // 4
        g = core % 4
        cols = np.r_[256 * g:256 * g + 256,
                     1024 + 256 * g:1024 + 256 * g + 256,
                     2048 + 256 * g:2048 + 256 * g + 256]
        bc = b_attn[cols]
        in_maps.append({
            "xT": np.ascontiguousarray(x[b].T).astype(bf16),
            "W": np.ascontiguousarray(W_attn[:, cols]).astype(bf16),
            "bcols": np.ascontiguousarray(
                bc[0:512].reshape(4, 128).T).astype(np.float32),
            "bv": np.ascontiguousarray(bc[512:768].reshape(1, 256)).astype(bf16),
            "Wp": np.ascontiguousarray(
                W_proj[256 * g:256 * g + 256, :]).astype(bf16),
            "mask": mask_np,
        })
    return in_maps


def kernel(x, W_attn, b_attn, W_proj, b_proj, _trace=False, _trace_kwargs=None):
    x = np.asarray(x, np.float32)
    W_attn = np.asarray(W_attn, np.float32)
    b_attn = np.asarray(b_attn, np.float32)
    W_proj = np.asarray(W_proj, np.float32)
    b_proj = np.asarray(b_proj, np.float32)

    nc = _get_nc()
    in_maps = make_in_maps(x, W_attn, b_attn, W_proj, b_proj)
    res = run_bass_kernel_spmd(
        nc, in_maps, core_ids=list(range(N_CORES)), trace=_trace,
        **(_trace_kwargs or {}),
    )
    B = x.shape[0]
    out = np.zeros((B, T, C), np.float32)
    for core in range(N_CORES):
        b = core // 4
        out[b] += res.results[core]["outT"].T.astype(np.float32)
    out += b_proj[None, None, :]
    if _trace:
        kernel._last_results = res
    return out


if __name__ == "__main__":
    # smoke test: build only
    nc = build_kernel()
    print("built ok")


# revision 7
# speedup vs baseline: 1.0817x; 1.0342x over previous
"""Trainium2 Bass kernel for causal self-attention (dense transformer block attn).

Reference computation (per batch b):
    qkv = x @ W_attn + b_attn ; split into per-head Q, K, V (16 heads, hs=64)
    att = softmax(mask(Q K^T / sqrt(hs))) ; y = att @ V ; out = y @ W_proj + b_proj

Sharding (8 cores): data parallel on B (2) x tensor parallel on head groups
(4 groups of 4 heads, Megatron-style column/row split of W_attn / W_proj).
Each core computes a partial out^T [1024, 2048] (bf16); host sums the 4
partials per batch, adds b_proj and transposes.

Core kernel layout notes:
  - Everything on-chip is transposed: x^T, q/k^T ([feature, T]), scores are
    computed as S^T = K Q^T with k-positions on partitions so that the PV
    matmul needs no transposes (P^T is the moving operand, V natural the
    stationary).
  - V is produced directly in natural [key, feature] layout by swapping the
    matmul roles (stationary = x^T k-chunk, moving = W_v columns); its bias
    is a rank-1 matmul (ones[1,128] x bv[1,128]) prepended to the chain.
    This removes all PE transposes and their DVE evacuation copies.
  - Emission is flash-style (outer loop over 512-wide q blocks, inner over
    128-wide k chunks). The ACT exp() stream is the pacing resource, so the
    schedule works to never stall it: Q/K projections for block qb+1 and
    V-natural rounds run as PE fillers inside block qb, and PSUM pools are
    split (scores / qkv+proj / pv) so no iteration-boundary matmul ever
    waits on the previous iteration's DVE backlog (which also kept HAM
    re-throttling the PE clock).
  - Softmax denominator: the PV stationary is [V | ones] (or [ones | V]) so
    the complementary 64 psum partitions accumulate copies of
    sum_k P[q,k]; a single-row reciprocal_approx_fast + a DRAM partition
    broadcast bounce (on the otherwise idle gpsimd SWDGE queue) yields the
    per-q scale; one DVE multiply per head normalizes during evacuation.
  - exp() runs on ScalarE straight out of PSUM in wide [128, 2, <=512]
    instructions (two heads at once) to amortize the ~352-cycle ACT
    overhead.
"""

import numpy as np
import ml_dtypes

import concourse.bass as bass
import concourse.tile as tile
import concourse.mybir as mybir
from concourse import bacc
from concourse.bass_utils import run_bass_kernel_spmd

BF16 = mybir.dt.bfloat16
F32 = mybir.dt.float32
AF = mybir.ActivationFunctionType

T = 2048          # sequence length
C = 1024          # model dim
HPC = 4           # heads per core
HS = 64           # head size
NF = 3 * HPC * HS  # per-core qkv features (768)
N_CORES = 8
QB = 512          # q block (psum bank of f32)

bf16 = ml_dtypes.bfloat16


def build_kernel():
    nc = bacc.Bacc("TRN2", target_bir_lowering=False, debug=False)

    xT = nc.dram_tensor("xT", [C, T], BF16, kind="ExternalInput").ap()
    W = nc.dram_tensor("W", [C, NF], BF16, kind="ExternalInput").ap()
    bcols = nc.dram_tensor("bcols", [128, 4], F32, kind="ExternalInput").ap()
    bv = nc.dram_tensor("bv", [1, 256], BF16, kind="ExternalInput").ap()
    Wp = nc.dram_tensor("Wp", [HPC * HS, C], BF16, kind="ExternalInput").ap()
    mask = nc.dram_tensor("mask", [128, 128], BF16, kind="ExternalInput").ap()
    outT = nc.dram_tensor("outT", [C, T], BF16, kind="ExternalOutput").ap()

    with tile.TileContext(nc) as tc:
        _emit(nc, tc, xT, W, bcols, bv, Wp, mask, outT)
    nc.compile()
    return nc


def _emit(nc, tc, xT, W, bcols, bv, Wp, mask, outT):
    from contextlib import ExitStack

    ctx = ExitStack()
    consts = ctx.enter_context(tc.tile_pool(name="consts", bufs=1))
    pt_pool = ctx.enter_context(tc.tile_pool(name="pt", bufs=1))
    rt_pool = ctx.enter_context(tc.tile_pool(name="rt", bufs=2))
    osb_pool = ctx.enter_context(tc.tile_pool(name="osb", bufs=2))
    ps_s = ctx.enter_context(tc.tile_pool(name="ps_s", bufs=2, space="PSUM"))
    ps_sm = ctx.enter_context(tc.tile_pool(name="ps_sm", bufs=2, space="PSUM"))
    ps_pv = ctx.enter_context(tc.tile_pool(name="ps_pv", bufs=2, space="PSUM"))

    # ---------------- constant / input loads ----------------
    # x and W interleaved per c-chunk with x split in T halves so the first
    # q blocks are available early; big/first loads on the Sync HWDGE queue,
    # second x half on the Scalar HWDGE queue, small consts on gpsimd SWDGE.
    xT_v = xT.rearrange("(c p) t -> p c t", p=128)
    xT_t = consts.tile([128, 8, T], BF16, tag="xT", name="xT_t")
    W_v = W.rearrange("(c p) n -> p c n", p=128)
    W_t = consts.tile([128, 8, NF], BF16, tag="W", name="W_t")
    # strict priority order on one queue: W cols for Q/K-pair0 + x q-block 0
    # first (unblocks the exp stream ~5us in), then the rest of W, then x
    # blocks 1..3 in use order.
    for c in range(8):
        nc.sync.dma_start(out=W_t[:, c, 0:384], in_=W_v[:, c, 0:384])
        nc.sync.dma_start(out=xT_t[:, c, 0:QB], in_=xT_v[:, c, 0:QB])
    for c in range(8):
        nc.sync.dma_start(out=W_t[:, c, 384:NF], in_=W_v[:, c, 384:NF])
    for qq in range(1, 4):
        for c in range(8):
            nc.sync.dma_start(out=xT_t[:, c, qq * QB:(qq + 1) * QB],
                              in_=xT_v[:, c, qq * QB:(qq + 1) * QB])
    b_t = consts.tile([128, 4], F32, tag="b", name="b_t")
    nc.gpsimd.dma_start(out=b_t, in_=bcols)
    bv_t = consts.tile([1, 256], BF16, tag="bv", name="bv_t")
    nc.gpsimd.dma_start(out=bv_t, in_=bv)
    Wp_t = consts.tile([128, 2, C], BF16, tag="Wp", name="Wp_t")
    nc.gpsimd.dma_start(out=Wp_t, in_=Wp.rearrange("(k p) n -> p k n", p=128))
    mask_t = consts.tile([128, 128], BF16, tag="mask", name="mask_t")
    nc.gpsimd.dma_start(out=mask_t, in_=mask)

    # DRAM scratch for the reciprocal broadcast bounce: [pair*qb, hl, 512]
    scr = nc.dram_tensor("pv_scr", [8, 2, QB], F32).ap()

    qkvT = consts.tile([128, 4, T], BF16, tag="qkvT", name="qkvT")
    # vnat[p, pair, jc, hl, col]: PV stationary tiles. hl=0: [V | ones],
    # hl=1: [ones | V] so that y lands on the partitions matching yT layout.
    vnat = consts.tile([128, 2, 16, 2, 128], BF16, tag="vnat", name="vnat")
    yT = consts.tile([128, 2, T], BF16, tag="yT", name="yT")
    ones1 = consts.tile([1, 128], BF16, tag="ones1", name="ones1")
    nc.vector.memset(ones1, 1.0)

    # warm up the ACT exp table early so the ~2.7us load overlaps the lead-in
    warm = consts.tile([128, 8], F32, tag="warm", name="warm")
    nc.vector.memset(warm, 0.0)
    nc.scalar.activation(warm, warm, AF.Exp, scale=1.0)

    # input-DMA-independent junk matmuls: keep the PE array fed during the
    # initial input-streaming window so HAM un-throttles before real work.
    # The junk psum shares the "pv" slots: all junk writes are emitted in
    # the first iteration, before any pv tile cycles onto its slot.
    jw = consts.tile([128, QB], BF16, tag="jw", name="jw")
    nc.vector.memset(jw, 0.0)
    junk = ps_pv.tile([128, QB], F32, tag="pv", name="junk")

    def keep_warm(n=2):
        for _ in range(n):
            nc.tensor.matmul(junk, lhsT=jw[:, 0:128], rhs=jw, start=True,
                             stop=True)

    nc.vector.memset(vnat[:, :, :, 0, 64:128], 1.0)
    nc.vector.memset(vnat[:, :, :, 1, 0:64], 1.0)

    # ---------------- phase helpers ----------------
    def qkv_part(nf, qb4):
        # one q block of q/k^T[nf*128:(nf+1)*128, :]  (+ bias on evac)
        ps = ps_sm.tile([128, QB], F32, tag="sm", name="ps_qkv")
        for c in range(8):
            nc.tensor.matmul(
                ps,
                lhsT=W_t[:, c, nf * 128:(nf + 1) * 128],
                rhs=xT_t[:, c, qb4 * QB:(qb4 + 1) * QB],
                start=(c == 0),
                stop=(c == 7),
            )
        nc.vector.tensor_scalar_add(
            qkvT[:, nf, qb4 * QB:(qb4 + 1) * QB], ps, b_t[:, nf:nf + 1]
        )

    def vnat_round(p, kc):
        # V natural for pair p, key chunk kc: [key, (hl0 hs | hl1 hs)].
        # rank-1 bias matmul + 8 c-chunk matmuls, then one strided copy
        # psum -> vnat[:, p, kc, hl, 64*hl : 64*hl+64].
        ps = ps_sm.tile([128, 128], F32, tag="sm", name="ps_vn")
        nc.tensor.matmul(ps, lhsT=ones1, rhs=bv_t[0:1, 128 * p:128 * p + 128],
                         start=True, stop=False)
        for c in range(8):
            nc.tensor.matmul(
                ps,
                lhsT=xT_t[:, c, kc * 128:(kc + 1) * 128],
                rhs=W_t[:, c, 512 + 128 * p:512 + 128 * p + 128],
                start=False,
                stop=(c == 7),
            )
        v0 = vnat[:, p, kc, 0, 0:64]
        dst = bass.AP(tensor=v0.tensor, offset=v0.offset,
                      ap=[v0.ap[0], [192, 2], [1, 64]])
        s0 = ps[:, 0:64]
        src = bass.AP(tensor=s0.tensor, offset=s0.offset,
                      ap=[s0.ap[0], [64, 2], [1, 64]])
        nc.vector.tensor_copy(dst, src)

    pt_tiles = {}

    def s_part(p, j, qb4):
        # scores^T + exp for pair p, key chunk j, q block qb4 (both heads)
        wj = T - 128 * j
        if (p, j) not in pt_tiles:
            pt_tiles[(p, j)] = pt_pool.tile(
                [128, 2, wj], BF16, tag=f"pt{j}",
                name=f"pt_{p}_{j}", bufs=2 if j < 2 else 1)
        pt = pt_tiles[(p, j)]
        qlo = max(128 * j, QB * qb4)
        qhi = QB * (qb4 + 1)
        lo = qlo - QB * qb4
        ps = ps_s.tile([128, 2, QB], F32, tag="s", name="ps_s_t")
        for hl in range(2):
            nc.tensor.matmul(
                ps[:, hl, lo:QB],
                lhsT=qkvT[64 * hl:64 * hl + 64, 2 + p, j * 128:(j + 1) * 128],
                rhs=qkvT[64 * hl:64 * hl + 64, p, qlo:qhi],
                start=True,
                stop=True,
            )
        nc.scalar.activation(
            pt[:, :, (qlo - 128 * j):(qhi - 128 * j)],
            ps[:, :, lo:QB],
            AF.Exp,
            scale=0.125,
        )
        if j // 4 == qb4:
            # zero the q < k upper triangle of the diagonal chunk (both heads
            # in one mul via a broadcast AP over the head dim)
            mb = bass.AP(tensor=mask_t.tensor, offset=mask_t.offset,
                         ap=[mask_t.ap[0], [0, 2], [1, 128]])
            nc.vector.tensor_mul(pt[:, :, 0:128], pt[:, :, 0:128], mb)

    sb_tiles = {}
    rt2_tiles = {}
    pv_ps = {}

    def pv_mms(p, hl, qb4, jlo, jhi, start, stop):
        ps = pv_ps[(p, hl)]
        for jp in range(jlo, jhi + 1):
            pt = pt_tiles[(p, jp)]
            qlo = max(qb4 * QB, 128 * jp)
            qhi = qb4 * QB + QB
            nc.tensor.matmul(
                ps[:, (qlo - qb4 * QB):(qhi - qb4 * QB)],
                lhsT=vnat[:, p, jp, hl, :],
                rhs=pt[:, hl, (qlo - 128 * jp):(qhi - 128 * jp)],
                start=(start and jp == jlo),
                stop=(stop and jp == jhi),
            )

    def pv_rect(p, qb4):
        # below-diagonal part of both heads' PV chains: reads only pt data
        # from earlier iterations, so it can go first in the iteration with
        # no fresh cross-engine deps. Leaves the psum accumulation open.
        for hl in range(2):
            pv_ps[(p, hl)] = ps_pv.tile([128, QB], F32, tag="pv",
                                        name=f"ps_pv{p}{hl}")
        if qb4 > 0:
            for hl in range(2):
                pv_mms(p, hl, qb4, 0, 4 * qb4 - 1, start=True, stop=False)

    def pv_diag(p, qb4):
        # diagonal-block chunks (their exp+mask land late in the previous
        # iteration's ACT/DVE queues, so this pops a few score steps in),
        # then evac + single-row scratch dump + the partition-broadcast
        # read-back for the pair (rt2 rows 0:64 = hl0, 64:128 = hl1).
        uid = p * 4 + qb4
        for hl in range(2):
            pv_mms(p, hl, qb4, 4 * qb4, 4 * qb4 + 3,
                   start=(qb4 == 0), stop=True)
            drow = 64 - 64 * hl  # one representative denominator-copy row
            sb = rt_pool.tile([128, QB], F32, tag="sb", name="sb", bufs=6)
            nc.vector.tensor_copy(sb, pv_ps.pop((p, hl)))
            nc.gpsimd.dma_start(out=scr[uid, hl, :], in_=sb[drow:drow + 1, :])
            sb_tiles[(p, hl, qb4)] = sb
        s1 = scr[uid]
        src = bass.AP(tensor=s1.tensor, offset=s1.offset,
                      ap=[[QB, 2], [0, 64], [1, QB]])
        rt2 = rt_pool.tile([128, QB], F32, tag="rt2", name="rt2", bufs=3)
        # the 256KB broadcast must go on a HWDGE queue: the single-ring
        # gpsimd SWDGE queue takes ~7us for this transfer vs ~1us fanned out
        nc.sync.dma_start(out=rt2, in_=src)
        rt2_tiles[(p, qb4)] = rt2

    def pv_norm(p, qb4):
        # normalize both heads' y into yT. The reciprocal happens here, one
        # emission block after pv2 issued the broadcast DMA, so the in-order
        # DVE queue never stalls waiting for it (full-tile approx covers
        # both heads at once; single-partition reciprocal_approx is broken).
        qsl = slice(qb4 * QB, (qb4 + 1) * QB)
        rt2 = rt2_tiles.pop((p, qb4))
        rc = rt_pool.tile([128, QB], F32, tag="rc", name="rc", bufs=2)
        nc.vector.reciprocal_approx_fast(out=rc, in_=rt2)
        for hl in range(2):
            ysl = slice(64 * hl, 64 * hl + 64)
            sb = sb_tiles.pop((p, hl, qb4))
            nc.vector.tensor_mul(yT[ysl, p, qsl], sb[ysl, :], rc[ysl, :])

    outT_v = outT.rearrange("(n p) t -> p n t", p=128)

    def proj_u(qb4, nf2, evac_engine):
        # final projection, one nf2 unit (2 psum rounds + output DMA) of the
        # 4 per q block (needs yT of both pairs for this block)
        qsl = slice(qb4 * QB, (qb4 + 1) * QB)
        ob = osb_pool.tile([128, 2, QB], BF16, tag="osb", name="ob")
        for sub in range(2):
            nf = nf2 * 2 + sub
            ps = ps_sm.tile([128, QB], F32, tag="sm", name="ps_o")
            for kc in range(2):
                nc.tensor.matmul(
                    ps,
                    lhsT=Wp_t[:, kc, nf * 128:(nf + 1) * 128],
                    rhs=yT[:, kc, qsl],
                    start=(kc == 0),
                    stop=(kc == 1),
                )
            if evac_engine == "scalar":
                nc.scalar.copy(ob[:, sub, :], ps)
            else:
                nc.vector.tensor_copy(ob[:, sub, :], ps)
        nc.sync.dma_start(out=outT_v[:, nf2 * 2:nf2 * 2 + 2, qsl], in_=ob)

    def proj_qb(qb4, evac_engine):
        for nf2 in range(4):
            proj_u(qb4, nf2, evac_engine)

    # ---------------- emission schedule ----------------
    # flash-style: per 512-wide q block of pair 0 then pair 1: scores+exp
    # for all k chunks <= the diagonal, with carry-over work (lagged
    # PV rect/diag, 2-blocks-lagged normalize, proj) and LOOK-AHEAD work
    # (next block's Q/K projections, V-natural rounds) popped at explicit
    # score steps. Rules encoded here:
    #   - pv_rect(prev) at step 0 (no fresh deps), pv_diag(prev) ~step 3
    #     (its exp+mask retire from the previous iteration's queues by then)
    #   - pv_norm(2-ago) at step 0 so its DVE muls land EARLY in the queue
    #     (its broadcast DMA has been in flight since mid-prev iteration)
    #     and proj of that block can follow in the same iteration.
    #   - Q/K(next) late; their DVE bias-adds still clear before the next
    #     iteration's first score step needs them.
    def iteration(p, qb4, fillers, warm_every=0):
        fill = sorted(fillers, key=lambda sf: sf[0])
        nf_s = 4 * qb4 + 4
        for j in range(nf_s):
            s_part(p, j, qb4)
            if warm_every:
                keep_warm(warm_every)
            while fill and fill[0][0] <= j:
                fill.pop(0)[1]()
        for _, f in fill:
            f()

    def F(fn, *a):
        return lambda: fn(*a)

    def vns(p, k0, steps):
        return [(s, F(vnat_round, p, k0 + i)) for i, s in enumerate(steps)]

    QK = qkv_part
    with nc.named_scope("p0"):
        qkv_part(0, 0)   # Q_0 block 0
        qkv_part(2, 0)   # K_0 chunks 0..3
        iteration(0, 0, vns(0, 0, [1, 1, 2, 2])
                  + [(2, F(QK, 0, 1)), (3, F(QK, 2, 1))], warm_every=2)
        iteration(0, 1, [(0, F(pv_rect, 0, 0)), (3, F(pv_diag, 0, 0))]
                  + vns(0, 4, [1, 2, 4, 5])
                  + [(5, F(QK, 0, 2)), (6, F(QK, 2, 2))])
        iteration(0, 2, [(0, F(pv_rect, 0, 1)), (0, F(pv_norm, 0, 0)),
                         (3, F(pv_diag, 0, 1))]
                  + vns(0, 8, [2, 4, 6, 8])
                  + [(9, F(QK, 0, 3)), (10, F(QK, 2, 3))])
        iteration(0, 3, [(0, F(pv_rect, 0, 2)), (0, F(pv_norm, 0, 1)),
                         (3, F(pv_diag, 0, 2))]
                  + vns(0, 12, [2, 5, 7, 9])
                  + [(11, F(QK, 1, 0)), (12, F(QK, 3, 0))])
    with nc.named_scope("p1"):
        iteration(1, 0, [(0, F(pv_rect, 0, 3)), (0, F(pv_norm, 0, 2)),
                         (2, F(pv_diag, 0, 3))]
                  + vns(1, 0, [1, 1, 2, 3])
                  + [(3, F(QK, 1, 1)), (3, F(QK, 3, 1))])
        iteration(1, 1, [(0, F(pv_rect, 1, 0)), (0, F(pv_norm, 0, 3)),
                         (3, F(pv_diag, 1, 0))]
                  + vns(1, 4, [1, 2, 4, 5])
                  + [(5, F(QK, 1, 2)), (6, F(QK, 3, 2))])
        iteration(1, 2, [(0, F(pv_rect, 1, 1)), (0, F(pv_norm, 1, 0)),
                         (3, F(pv_diag, 1, 1))]
                  + vns(1, 8, [2, 4, 6, 8])
                  + [(5, F(proj_u, 0, 0, "vector")),
                     (6, F(proj_u, 0, 1, "vector")),
                     (7, F(proj_u, 0, 2, "vector")),
                     (8, F(proj_u, 0, 3, "vector")),
                     (9, F(QK, 1, 3)), (10, F(QK, 3, 3))])
        iteration(1, 3, [(0, F(pv_rect, 1, 2)), (0, F(pv_norm, 1, 1)),
                         (3, F(pv_diag, 1, 2))]
                  + vns(1, 12, [2, 5, 8, 11])
                  + [(6, F(proj_u, 1, 0, "vector")),
                     (7, F(proj_u, 1, 1, "vector")),
                     (9, F(proj_u, 1, 2, "vector")),
                     (10, F(proj_u, 1, 3, "vector")),
                     (13, F(pv_norm, 1, 2))])
    with nc.named_scope("tail"):
        pv_rect(1, 3)
        with nc.named_scope("prj2"):
            proj_u(2, 0, "scalar")
            proj_u(2, 1, "scalar")
        pv_diag(1, 3)
        with nc.named_scope("prj2b"):
            proj_u(2, 2, "scalar")
            proj_u(2, 3, "scalar")
        pv_norm(1, 3)
        with nc.named_scope("prj3"):
            proj_qb(3, "scalar")
    ctx.close()


# ---------------------------------------------------------------------------
# host-side wrapper
# ---------------------------------------------------------------------------

_NC_CACHE = {}


def _get_nc():
    if "nc" not in _NC_CACHE:
        _NC_CACHE["nc"] = build_kernel()
    return _NC_CACHE["nc"]


def make_in_maps(x, W_attn, b_attn, W_proj, b_proj):
    # multiplicative causal mask for the diagonal chunk, [k, q]: 1 where q >= k
    mask_np = np.triu(np.ones((128, 128), np.float32)).astype(bf16)
    in_maps = []
    for core in range(N_CORES):
        b = core // 4
        g = core % 4
        cols = np.r_[256 * g:256 * g + 256,
                     1024 + 256 * g:1024 + 256 * g + 256,
                     2048 + 256 * g:2048 + 256 * g + 256]
        bc = b_attn[cols]
        in_maps.append({
            "xT": np.ascontiguousarray(x[b].T).astype(bf16),
            "W": np.ascontiguousarray(W_attn[:, cols]).astype(bf16),
            "bcols": np.ascontiguousarray(
                bc[0:512].reshape(4, 128).T).astype(np.float32),
            "bv": np.ascontiguousarray(bc[512:768].reshape(1, 256)).astype(bf16),
            "Wp": np.ascontiguousarray(
                W_proj[256 * g:256 * g + 256, :]).astype(bf16),
            "mask": mask_np,
        })
    return in_maps


def kernel(x, W_attn, b_attn, W_proj, b_proj, _trace=False, _trace_kwargs=None):
    x = np.asarray(x, np.float32)
    W_attn = np.asarray(W_attn, np.float32)
    b_attn = np.asarray(b_attn, np.float32)
    W_proj = np.asarray(W_proj, np.float32)
    b_proj = np.asarray(b_proj, np.float32)

    nc = _get_nc()
    in_maps = make_in_maps(x, W_attn, b_attn, W_proj, b_proj)
    res = run_bass_kernel_spmd(
        nc, in_maps, core_ids=list(range(N_CORES)), trace=_trace,
        **(_trace_kwargs or {}),
    )
    B = x.shape[0]
    out = np.zeros((B, T, C), np.float32)
    for core in range(N_CORES):
        b = core // 4
        out[b] += res.results[core]["outT"].T.astype(np.float32)
    out += b_proj[None, None, :]
    if _trace:
        kernel._last_results = res
    return out


if __name__ == "__main__":
    # smoke test: build only
    nc = build_kernel()
    print("built ok")
